# revision 14
# baseline (speedup 1.0000x reference)
"""Trainium2 Bass kernel for GPT-2 style attention block (B=2, S=2048, D=1024, H=16).

Sharding (8 cores): data-parallel over batch (2) x tensor-parallel over heads (4 per
core). Each core: QKV projection for its 4 heads over the full sequence, full-seq
causal attention (transposed-scores layout: softmax reduction folded into the PV
matmul via a ones-column in V), then a row-parallel partial c_proj over the full
sequence using only this core's 256 rows of c_proj_w. No collectives: the host
sums the 4 per-head-group partials per batch (plus the folded v-bias term), so
each core's span is pure compute with no cross-core sync.

Schedule: work is organized in per-qt rounds (512 query columns each). The q/k
projections for query-block qt and key-block qt plus the V pieces for that key
range are emitted inside the round, so the tensor engine always has dense matmul
work to overlap the softmax exps (scalar engine) and stays HAM-warm.

Causal structure: score blocks on the diagonal are shrunk to their unmasked
query range and packed contiguously in PSUM so the exp covers no dead columns.
Masking is post-exp: GpSimd multiplies the probability tile by a 0/1 triangle
in SBUF (keeps DVE and the PSUM ports free). Softmax normalization is off-PE:
reciprocal on DVE, partition broadcast on GpSimd, fused scale-multiply on DVE.

Compute dtype bf16 (fp32 PSUM accumulation); normalization in fp32; partial
outputs shipped as fp16 to halve DMA.
"""
import sys
sys.path.insert(0, '/opt/trn_rl_repo')

import numpy as np
import ml_dtypes

import concourse.bass as bass
import concourse.mybir as mybir
import concourse.tile as tile
from concourse import bacc
from concourse.bass_utils import run_bass_kernel_spmd

B, S, D = 2, 2048, 1024
H, HD = 16, 64
NCORES = 8
HPC = H // 4          # heads per core = 4

F32 = mybir.dt.float32
F16 = mybir.dt.float16
BF16 = mybir.dt.bfloat16
ADD = mybir.AluOpType.add
MULT = mybir.AluOpType.mult
EXP = mybir.ActivationFunctionType.Exp


def _emit(nc, tc):
    xT = nc.dram_tensor("xT", [D, S], BF16, kind="ExternalInput").ap()
    w_qk = nc.dram_tensor("w_qk", [D, 512], BF16, kind="ExternalInput").ap()
    w_v = nc.dram_tensor("w_v", [D, 256], BF16, kind="ExternalInput").ap()
    w_p = nc.dram_tensor("w_p", [256, D], BF16, kind="ExternalInput").ap()
    bqk = nc.dram_tensor("bqk", [128, 4], F32, kind="ExternalInput").ap()
    cmask = nc.dram_tensor("cmask", [128, 128], BF16, kind="ExternalInput").ap()
    out = nc.dram_tensor("out", [S, D], F16, kind="ExternalOutput").ap()

    from contextlib import ExitStack
    ctx = ExitStack()
    cst = ctx.enter_context(tc.tile_pool(name="cst", bufs=1))
    pw = ctx.enter_context(tc.tile_pool(name="pw", bufs=2, space="PSUM"))
    pat = ctx.enter_context(tc.tile_pool(name="pat", bufs=2, space="PSUM"))
    psc = ctx.enter_context(tc.tile_pool(name="psc", bufs=2, space="PSUM"))
    sb = ctx.enter_context(tc.tile_pool(name="sb", bufs=3))

    # ---- resident SBUF loads, split across both HWDGE queues (sync + scalar)
    # and ordered so round 0's operands land first ----
    xT_sb = cst.tile([128, 8, S], BF16)
    wqk_sb = cst.tile([128, 8, 512], BF16)
    wv_sb = cst.tile([128, 8, 256], BF16)
    bqk_sb = cst.tile([128, 4], F32)
    cm_sb = cst.tile([128, 128], BF16)
    wp_sb = cst.tile([128, 2, D], BF16)
    nc.scalar.dma_start(bqk_sb[:], bqk)
    nc.scalar.dma_start(cm_sb[:], cmask)
    for k in range(8):
        nc.scalar.dma_start(wqk_sb[:, k],
                            w_qk.rearrange("(k p) n -> p k n", p=128)[:, k])
    for k in range(8):
        nc.scalar.dma_start(wv_sb[:, k],
                            w_v.rearrange("(k p) n -> p k n", p=128)[:, k])
    nc.scalar.dma_start(wp_sb[:], w_p.rearrange("(k p) n -> p k n", p=128))
    for qt in range(4):
        for k in range(8):
            nc.sync.dma_start(
                xT_sb[:, k, qt * 512:(qt + 1) * 512],
                xT.rearrange("(k p) n -> p k n", p=128)[:, k, qt * 512:(qt + 1) * 512])

    # PE warmer: dependency-free junk matmuls keep the array busy during the
    # input DMAs so HAM unthrottles before real work arrives
    ones_sb = cst.tile([1, 64], BF16)
    nc.vector.memset(ones_sb[:], 1.0)
    wrow = sb.tile([1, 512], BF16, tag="wrow")
    nc.vector.memset(wrow[:], 1.0)
    warm_ps = pw.tile([128, 512], F32, tag="w", name="warm")
    for _ in range(16):
        nc.tensor.matmul(warm_ps[0:64, :], ones_sb[:], wrow[:],
                         start=True, stop=True)

    # qkT [512, 2048]: rows 0-255 = q^T (4 heads x 64, prescaled 1/8), 256-511 = k^T
    qkT_sb = cst.tile([128, 4, S], BF16)

    def qk_proj(m, qt):
        # q^T (m=0,1) / k^T (m=2,3) for one 512-column sequence block
        ps = pw.tile([128, 512], F32, tag="w", name=f"qk{m}_{qt}")
        for k in range(8):
            nc.tensor.matmul(
                ps[:], wqk_sb[:, k, m * 128:(m + 1) * 128],
                xT_sb[:, k, qt * 512:(qt + 1) * 512],
                start=(k == 0), stop=(k == 7))
        nc.vector.tensor_scalar(
            out=qkT_sb[:, m, qt * 512:(qt + 1) * 512], in0=ps[:],
            scalar1=bqk_sb[:, m:m + 1], scalar2=None, op0=ADD)

    # V with interleaved ones column: V_sb [128, 16, 4*65]
    V_sb = cst.tile([128, 16, HPC * 65], BF16)

    def v_ones():
        nc.vector.memset(
            V_sb[:].rearrange("p m (h c) -> p m h c", c=65)[:, :, :, 64:65], 1.0)

    def v_piece(m):
        ps = pw.tile([128, 512], F32, tag="w", name=f"v{m}")
        for k in range(8):
            nc.tensor.matmul(
                ps[:, :256], xT_sb[:, k, m * 128:(m + 1) * 128], wv_sb[:, k, :],
                start=(k == 0), stop=(k == 7))
        nc.vector.tensor_copy(
            out=V_sb[:, m].rearrange("p (h c) -> p h c", c=65)[:, :, 0:64],
            in_=ps[:, :256].rearrange("p (h c) -> p h c", c=64))

    attnT_sb = cst.tile([128, 2, S], BF16)

    def attend_pair(j, qt):
        # heads 2j (partitions 0-63) and 2j+1 (64-127) interleaved: their K=64
        # score matmuls auto-derive different PE row-groups from base_partition
        # and run concurrently when adjacent in the queue
        sub = j
        at = {0: pat.tile([128, 512], F32, tag="at", name=f"atA{j}_{qt}"),
              64: pat.tile([128, 512], F32, tag="at", name=f"atB{j}_{qt}")}
        nkb = 4 * qt + 4
        for g0 in range(0, nkb, 2):
            gl = list(range(g0, min(g0 + 2, nkb)))
            rels = [max(0, kb * 128 - qt * 512) for kb in gl]
            offs = [0, 512 - rels[0]] if len(gl) == 2 else [0]
            w = offs[-1] + 512 - rels[-1]
            sc = {po: psc.tile([128, 1024], F32, tag="sc", name=f"sc{j}_{qt}_{g0}_{po}")
                  for po in (0, 64)}
            for i, kb in enumerate(gl):
                for po in (0, 64):
                    nc.tensor.matmul(
                        sc[po][:, offs[i]:offs[i] + 512 - rels[i]],
                        qkT_sb[po:po + 64, 2 + sub, kb * 128:(kb + 1) * 128],
                        qkT_sb[po:po + 64, sub, qt * 512 + rels[i]:(qt + 1) * 512],
                        start=True, stop=True)
            pt = {po: sb.tile([128, 1024], BF16, tag="pt", name=f"pt{po}")
                  for po in (0, 64)}
            for po in (0, 64):
                nc.scalar.activation(out=pt[po][:, :w], in_=sc[po][:, :w], func=EXP)
                for i, kb in enumerate(gl):
                    if kb * 128 >= qt * 512:  # post-exp triangle zeroing
                        nc.vector.tensor_tensor(
                            pt[po][:, offs[i]:offs[i] + 128],
                            pt[po][:, offs[i]:offs[i] + 128], cm_sb[:], MULT)
            for i, kb in enumerate(gl):
                for po in (0, 64):
                    h = 2 * j + (po > 0)
                    nc.tensor.matmul(
                        at[po][0:65, rels[i]:512], V_sb[:, kb, h * 65:(h + 1) * 65],
                        pt[po][:, offs[i]:offs[i] + 512 - rels[i]],
                        start=(kb == 0), stop=(kb == nkb - 1))
        # normalize off-PE: denominator to SBUF, 1/x (DVE), broadcast partition
        # 0 -> 64 (GpSimd), fused scale-multiply PSUM->SBUF (DVE)
        for po in (0, 64):
            den1 = sb.tile([1, 512], F32, tag="den1")
            nc.vector.tensor_copy(out=den1[:], in_=at[po][64:65, :])
            rec1 = sb.tile([1, 512], F32, tag="rec1")
            nc.vector.reciprocal_approx_fast(rec1[:], den1[:])
            recb = sb.tile([64, 512], F32, tag="recb")
            nc.gpsimd.partition_broadcast(recb[:], rec1[:])
            sl = attnT_sb[po:po + 64, sub, qt * 512:(qt + 1) * 512]
            nc.vector.tensor_tensor(sl, at[po][0:64, :], recb[:], MULT)

    def c_proj(ms, tail=False):
        # partial c_proj: contract only this core's 256 D-rows (2 u-blocks of
        # 128), full 2048-seq output; host sums partials across head groups.
        # In the tail (scalar engine idle) the PSUM evacuation is split across
        # Vector and Scalar and the out-DMA goes on the second queue.
        for m in ms:
            out_sb = sb.tile([128, D], F16, tag="out")
            ps = [pw.tile([128, 512], F32, tag="w", name=f"pj{m}_{n}") for n in range(2)]
            for u in range(2):
                for n in range(2):
                    nc.tensor.matmul(
                        ps[n][:], attnT_sb[:, u, m * 128:(m + 1) * 128],
                        wp_sb[:, u, n * 512:(n + 1) * 512],
                        start=(u == 0), stop=(u == 1))
            nc.vector.tensor_copy(out=out_sb[:, 0:512], in_=ps[0][:])
            if tail:
                nc.scalar.activation(out=out_sb[:, 512:1024], in_=ps[1][:],
                                     func=mybir.ActivationFunctionType.Copy)
                nc.scalar.dma_start(out[m * 128:(m + 1) * 128, :], out_sb[:])
            else:
                nc.vector.tensor_copy(out=out_sb[:, 512:1024], in_=ps[1][:])
                nc.sync.dma_start(out[m * 128:(m + 1) * 128, :], out_sb[:])

    # ---- per-qt rounds: just-in-time projections keep the PE stream dense.
    # The first head pair launches as soon as its q/k blocks exist so the
    # scalar engine starts exp-ing early; v-pieces and prev-round c_proj
    # blocks are spread between the pairs as PE filler for exp-gated
    # stretches ----
    v_ones()
    for qt in range(4):
        qk_proj(0, qt)
        qk_proj(2, qt)
        for m in range(4 * qt, 4 * qt + 4):
            v_piece(m)              # V for this round's diagonal key blocks
        attend_pair(0, qt)
        qk_proj(1, qt)
        qk_proj(3, qt)
        if qt:
            c_proj((4 * qt - 4, 4 * qt - 3))
        attend_pair(1, qt)
        if qt:
            c_proj((4 * qt - 2, 4 * qt - 1))
    c_proj(tuple(range(12, 16)), tail=True)

    ctx.close()


def build_nc():
    nc = bacc.Bacc("TRN2", target_bir_lowering=False, debug=False, num_devices=NCORES)
    with tile.TileContext(nc) as tc:
        _emit(nc, tc)
    nc.compile()
    return nc


def shard_inputs(hidden_states, c_attn_w, c_attn_b, c_proj_w, c_proj_b):
    x = np.asarray(hidden_states, np.float32)
    W = np.asarray(c_attn_w, np.float32)
    bqkv = np.asarray(c_attn_b, np.float32)
    Wp = np.asarray(c_proj_w, np.float32)

    wq, wk, wv = W[:, :D] * 0.125, W[:, D:2 * D], W[:, 2 * D:]
    bq, bk = bqkv[:D] * 0.125, bqkv[D:2 * D]

    # 128x128 causal triangle keep-mask: 0 where key (row) > query (col), else 1
    k_i = np.arange(128)[:, None]
    q_i = np.arange(128)[None, :]
    cm = (k_i <= q_i).astype(ml_dtypes.bfloat16)

    in_maps = []
    for c in range(NCORES):
        b, r = divmod(c, 4)
        hs = slice(256 * r, 256 * (r + 1))
        w_qk = np.concatenate([wq[:, hs], wk[:, hs]], axis=1)
        bqk_t = np.concatenate([bq[hs], bk[hs]]).reshape(4, 128).T.copy()
        in_maps.append(dict(
            xT=np.ascontiguousarray(x[b].T).astype(ml_dtypes.bfloat16),
            w_qk=w_qk.astype(ml_dtypes.bfloat16),
            w_v=wv[:, hs].astype(ml_dtypes.bfloat16),
            w_p=np.ascontiguousarray(Wp[hs, :]).astype(ml_dtypes.bfloat16),
            bqk=bqk_t.astype(np.float32),
            cmask=cm,
        ))
    return in_maps


def unshard(results, c_attn_b, c_proj_w, c_proj_b):
    bqkv = np.asarray(c_attn_b, np.float32)
    Wp = np.asarray(c_proj_w, np.float32)
    bp = np.asarray(c_proj_b, np.float32)
    # softmax rows sum to 1, so the v-bias passes through attention unchanged:
    # out = (softmax @ xWv + bv) @ Wp + bp = sum(partials) + bv@Wp + bp
    beff = (bqkv[2 * D:] @ Wp + bp).astype(np.float32)
    full = np.zeros((B, S, D), np.float32)
    for c in range(NCORES):
        b = c // 4
        full[b] += results[c]["out"].astype(np.float32)
    full += beff
    return full


_NC = None


def kernel(**inputs):
    global _NC
    if _NC is None:
        _NC = build_nc()
    in_maps = shard_inputs(**inputs)
    res = run_bass_kernel_spmd(_NC, in_maps, core_ids=list(range(NCORES)))
    return unshard(res.results, inputs["c_attn_b"], inputs["c_proj_w"],
                   inputs["c_proj_b"])


if __name__ == "__main__":
    import jax
    with jax.default_device(jax.devices("cpu")[0]):
        import reference
        inputs = {k: np.asarray(v) for k, v in reference.setup_inputs().items()}
        expected = np.asarray(reference.reference(**inputs))
    actual = kernel(**inputs)
    err = np.abs(actual - expected)
    print("max abs err:", err.max(), "rel:", err.max() / np.abs(expected).max())
